# revision 1
# baseline (speedup 1.0000x reference)
"""KNN-impute kernel (nn_CalcImpute) for Trainium2, 8 NeuronCores.

Computation (see reference): for each of 8192 receiver rows, find the 16
smallest entries of a 50000-wide distance row (ties -> lowest column index,
matching jax.lax.top_k), gather fit_X_col at those columns, and output the
mean of the valid (mask==0) donor values (0 if none valid).

Sharding: pure data parallel over rows; each of the 8 cores gets 1024 rows.
fit/mask-derived tables are tiny and replicated.

Device algorithm per 128-row tile (rows live in partitions):
  P1  stream the 50000 columns in panels, segmented min (seg=50) ->
      1000 segment minima per row.  (the only full pass over the data)
  P2  negate seg-mins; 4 rounds of max8/max_index/match_replace give the
      24 segments with the smallest minima per row (+ the 25th min for a
      coverage flag).  All top-16 elements provably live in the 16 segs
      with smallest minima, so 24 gives slack.
  P3  indirect-DMA gather of those 24 segments (24x50 f32) per row from
      DRAM, plus the matching [G; V] table slices (G = fitX * valid,
      V = valid, precomputed on host).
  P4  negate candidates; 2x(max8+match_replace) marks the 16 smallest
      (ties by scan position); selection mask = (orig != replaced);
      numerator/denominator via fused multiply-reduce; res = num/den
      (den==0 -> den=1).  A 3rd max8 gives the 17th value for the
      tie-ambiguity flag.

Exactness: match_replace/max_index replace/report first occurrences, so the
selected *set* equals jax.lax.top_k's except when the 16th and 17th smallest
values are exactly equal (~0.3% of rows) or segment coverage is ambiguous.
Those rows are flagged on device and recomputed exactly on host.
"""

import os
import sys

for _p in ("/opt/trn_rl_repo", "/root/.axon_site/_ro/trn_rl_repo"):
    if os.path.isdir(_p) and _p not in sys.path:
        sys.path.insert(0, _p)

import numpy as np

import concourse.bass as bass
import concourse.bacc as bacc_mod
import concourse.mybir as mybir
import concourse.tile as tile
from concourse.bass_utils import run_bass_kernel_spmd

N_CORES = 8
R_TOTAL = 8192
N = 50000
P = 128              # SBUF partitions
S = 50               # segment size for the min prefilter
NSEG = N // S        # 1000 segments per row
PC = 10000           # panel columns streamed per DMA
NPAN = N // PC       # 4 panels
NSEG_P = PC // S     # 250 segments per panel
KSEG = 16            # candidate segments gathered per row
CAND = KSEG * S      # 1200 candidate values per row
NEG_BIG = -3.0e38    # replacement sentinel on the negated scale
F32 = mybir.dt.float32
U32 = mybir.dt.uint32


def build_bass(rows: int, repeat: int = 1):
    """Bass program for one core processing `rows` rows (multiple of 128).

    repeat>1 re-runs the whole pipeline (for slope-based benchmarking).
    """
    assert rows % P == 0
    nt = rows // P

    nc = bacc_mod.Bacc()
    dist = nc.dram_tensor("dist", [rows, N], F32, kind="ExternalInput")
    gv = nc.dram_tensor("gv", [NSEG, 2 * S], F32, kind="ExternalInput")
    out_res = nc.dram_tensor("res", [P, nt], F32, kind="ExternalOutput")
    out_flag = nc.dram_tensor("flag", [P, nt], F32, kind="ExternalOutput")

    # flat views for indirect gathers (offset must be 0)
    dist_flat = dist[:, :].rearrange("r (s e) -> (r s) e", e=S)
    gv_flat = gv[:, :]

    with tile.TileContext(nc) as tc:
        with (
            tc.tile_pool(name="panels", bufs=3) as pan_pool,
            tc.tile_pool(name="segs", bufs=2) as seg_pool,
            tc.tile_pool(name="small", bufs=2) as small_pool,
            tc.tile_pool(name="cands", bufs=2) as cand_pool,
            tc.tile_pool(name="persist", bufs=1) as persist_pool,
        ):
            res_sb = persist_pool.tile([P, nt], F32)
            flag_sb = persist_pool.tile([P, nt], F32)

            def emit_front(rt, p4_chunks=()):
                """P1 stream+segmin, P2 top-16 segments, P3 gathers.

                p4_chunks: closures of the previous tile's P4, interleaved
                one per panel so cross-engine handoffs hide behind the
                segmented reduces.
                """
                p4_chunks = list(p4_chunks)
                segmin = seg_pool.tile([P, NSEG], F32, tag="segmin")
                for pan in range(NPAN):
                    x = pan_pool.tile([P, PC], F32, tag="panel")
                    nc.sync.dma_start(
                        out=x,
                        in_=dist[rt * P:(rt + 1) * P, pan * PC:(pan + 1) * PC],
                    )
                    nc.vector.tensor_reduce(
                        out=segmin[:, pan * NSEG_P:(pan + 1) * NSEG_P],
                        in_=x.rearrange("p (s e) -> p s e", e=S),
                        axis=mybir.AxisListType.X,
                        op=mybir.AluOpType.min,
                    )
                    if p4_chunks:
                        p4_chunks.pop(0)()
                while p4_chunks:
                    p4_chunks.pop(0)()

                nsm = seg_pool.tile([P, NSEG], F32, tag="nsm")
                nc.scalar.mul(nsm, segmin, -1.0)
                segidx = small_pool.tile([P, KSEG], U32, tag="segidx")
                v_seg = small_pool.tile([P, 3, 8], F32, tag="v_seg")
                for rnd in range(2):
                    v8 = v_seg[:, rnd, :]
                    nc.vector.max(out=v8, in_=nsm)
                    nc.vector.max_index(
                        out=segidx[:, rnd * 8:(rnd + 1) * 8],
                        in_max=v8, in_values=nsm)
                    nc.vector.match_replace(
                        out=nsm, in_to_replace=v8, in_values=nsm,
                        imm_value=NEG_BIG)
                # 17th smallest seg-min (negated) for the coverage flag
                nc.vector.max(out=v_seg[:, 2, :], in_=nsm)

                # P3: gather candidate segments + G/V slices.
                rowbase = small_pool.tile([P, 1], U32, tag="rowbase")
                nc.gpsimd.iota(rowbase, pattern=[[0, 1]],
                               base=rt * P * NSEG, channel_multiplier=NSEG)
                off_dist = small_pool.tile([P, KSEG], U32, tag="off_dist")
                nc.vector.tensor_tensor(
                    out=off_dist, in0=segidx,
                    in1=rowbase.to_broadcast([P, KSEG]),
                    op=mybir.AluOpType.add)
                # HW SWDGE indirect gather is only reliable with one offset
                # per partition, so issue one gather per candidate column.
                cand = cand_pool.tile([P, KSEG, S], F32, tag="cand")
                gvc = cand_pool.tile([P, KSEG, 2 * S], F32, tag="gvc")
                for t in range(KSEG):
                    nc.gpsimd.indirect_dma_start(
                        out=cand[:, t, :], out_offset=None,
                        in_=dist_flat,
                        in_offset=bass.IndirectOffsetOnAxis(
                            ap=off_dist[:, t:t + 1], axis=0),
                    )
                    nc.gpsimd.indirect_dma_start(
                        out=gvc[:, t, :], out_offset=None,
                        in_=gv_flat,
                        in_offset=bass.IndirectOffsetOnAxis(
                            ap=segidx[:, t:t + 1], axis=0),
                    )
                return dict(rt=rt, cand=cand, gvc=gvc, v_seg=v_seg)

            def make_p4_chunks(st):
                """Exact top-16 + weighted mean, as 5 schedulable chunks."""
                rt, cand, gvc, v_seg = (st["rt"], st["cand"], st["gvc"],
                                        st["v_seg"])
                ncand = cand_pool.tile([P, CAND], F32, tag="ncand")
                ncandb = cand_pool.tile([P, CAND], F32, tag="ncandb")
                v_c = small_pool.tile([P, 3, 8], F32, tag="v_c")
                sel = cand_pool.tile([P, CAND], F32, tag="sel")
                junk = cand_pool.tile([P, CAND], F32, tag="junk")
                junk2 = cand_pool.tile([P, CAND], F32, tag="junk2")
                acc = small_pool.tile([P, 8], F32, tag="acc")
                num, den = acc[:, 0:1], acc[:, 1:2]
                sel3 = sel.rearrange("p (a b) -> p a b", b=S)
                junk3 = junk.rearrange("p (a b) -> p a b", b=S)

                def c1():
                    nc.scalar.mul(ncand,
                                  cand.rearrange("p a b -> p (a b)"), -1.0)
                    nc.vector.max(out=v_c[:, 0, :], in_=ncand)

                def c2():
                    nc.vector.match_replace(
                        out=ncandb, in_to_replace=v_c[:, 0, :],
                        in_values=ncand, imm_value=NEG_BIG)
                    nc.vector.max(out=v_c[:, 1, :], in_=ncandb)

                def c3():
                    nc.vector.match_replace(
                        out=ncandb, in_to_replace=v_c[:, 1, :],
                        in_values=ncandb, imm_value=NEG_BIG)
                    nc.vector.max(out=v_c[:, 2, :], in_=ncandb)
                    nc.vector.tensor_tensor(
                        out=sel, in0=ncand, in1=ncandb,
                        op=mybir.AluOpType.not_equal)

                def c4():
                    # (tensor_tensor_reduce crashes the exec unit on this
                    # HW; multiply on DVE, sum via ACT Copy-with-accum)
                    nc.vector.tensor_tensor(out=junk3, in0=sel3,
                                            in1=gvc[:, :, 0:S],
                                            op=mybir.AluOpType.mult)
                    nc.scalar.activation(
                        out=junk2, in_=junk,
                        func=mybir.ActivationFunctionType.Copy,
                        accum_out=num)
                    nc.vector.tensor_tensor(out=junk3, in0=sel3,
                                            in1=gvc[:, :, S:2 * S],
                                            op=mybir.AluOpType.mult)

                def c5():
                    nc.scalar.activation(
                        out=junk2, in_=junk,
                        func=mybir.ActivationFunctionType.Copy,
                        accum_out=den)
                    # denp = den + (den == 0)
                    denp, recip = acc[:, 3:4], acc[:, 4:5]
                    nc.vector.scalar_tensor_tensor(
                        out=denp, in0=den, scalar=0.0, in1=den,
                        op0=mybir.AluOpType.is_equal,
                        op1=mybir.AluOpType.add)
                    nc.vector.reciprocal(recip, denp)
                    nc.vector.tensor_mul(res_sb[:, rt:rt + 1], num, recip)
                    # flag = max(v17, m17_neg) >= v16 (negated scale):
                    # boundary-tie or ambiguous segment coverage
                    nc.vector.scalar_tensor_tensor(
                        out=flag_sb[:, rt:rt + 1], in0=v_c[:, 2, 0:1],
                        scalar=v_seg[:, 2, 0:1], in1=v_c[:, 1, 7:8],
                        op0=mybir.AluOpType.max, op1=mybir.AluOpType.is_ge)

                return [c1, c2, c3, c4, c5]

            # software pipeline: P4 of tile i is chunked and interleaved
            # into tile i+1's panel loop, hiding gather latency and
            # cross-engine handoffs behind the segmented reduces.
            pending = None
            for rt in [t for _ in range(repeat) for t in range(nt)]:
                pending = emit_front(rt, make_p4_chunks(pending)
                                     if pending else ())
            for c in make_p4_chunks(pending):
                c()

            nc.sync.dma_start(out=out_res[:, :], in_=res_sb)
            nc.sync.dma_start(out=out_flag[:, :], in_=flag_sb)

    nc.compile()
    return nc


def _host_reference_rows(dist_rows: np.ndarray, fit: np.ndarray,
                         mask: np.ndarray, k: int) -> np.ndarray:
    """Exact recompute (jax.lax.top_k tie semantics) for flagged rows."""
    out = np.empty(dist_rows.shape[0], dtype=np.float32)
    valid = (1 - mask).astype(np.float32)
    for i, row in enumerate(dist_rows):
        r = np.nan_to_num(row, nan=1e10)
        idx = np.argsort(r, kind="stable")[:k]
        w = valid[idx]
        ws = np.float32(w.sum(dtype=np.float32))
        div = ws if ws != 0 else np.float32(1.0)
        num = np.float32((fit[idx].astype(np.float32) * w).sum(dtype=np.float32))
        out[i] = num / div
    return out


def _prep_tables(fit_X_col: np.ndarray, mask_fit_X_col: np.ndarray):
    valid = (1 - mask_fit_X_col).astype(np.float32)
    g = fit_X_col.astype(np.float32) * valid
    gv_tab = np.empty((NSEG, 2, S), dtype=np.float32)
    gv_tab[:, 0, :] = g.reshape(NSEG, S)
    gv_tab[:, 1, :] = valid.reshape(NSEG, S)
    return gv_tab.reshape(NSEG, 2 * S)


def kernel(dist_pot_donors, n_neighbors, fit_X_col, mask_fit_X_col,
           _trace=False, _tmpdir=None):
    dist = np.ascontiguousarray(np.asarray(dist_pot_donors, dtype=np.float32))
    fit = np.asarray(fit_X_col, dtype=np.float32)
    mask = np.asarray(mask_fit_X_col)
    k = int(np.asarray(n_neighbors))
    assert dist.shape == (R_TOTAL, N) and k == 16, (dist.shape, k)

    gv_tab = _prep_tables(fit, mask)
    rows = R_TOTAL // N_CORES
    nt = rows // P

    nc = build_bass(rows)
    in_maps = [
        {"dist": dist[c * rows:(c + 1) * rows], "gv": gv_tab}
        for c in range(N_CORES)
    ]
    kw = {}
    if _trace:
        kw.update(trace=True, tmpdir=_tmpdir)
    br = run_bass_kernel_spmd(nc, in_maps, core_ids=list(range(N_CORES)), **kw)

    out = np.empty(R_TOTAL, dtype=np.float32)
    flags = np.empty(R_TOTAL, dtype=bool)
    for c, r in enumerate(br.results):
        # res[p, t] holds row c*rows + t*128 + p
        out[c * rows:(c + 1) * rows] = r["res"].T.reshape(rows)
        flags[c * rows:(c + 1) * rows] = r["flag"].T.reshape(rows) != 0

    n_flagged = int(flags.sum())
    if n_flagged:
        out[flags] = _host_reference_rows(dist[flags], fit, mask, k)
    kernel._last = {"exec_time_ns": br.exec_time_ns,
                    "mean_exec_time_ns": br.mean_exec_time_ns,
                    "n_flagged": n_flagged,
                    "trace": br.instructions_and_trace}
    return out



# revision 3
# speedup vs baseline: 1.1284x; 1.1284x over previous
"""KNN-impute kernel (nn_CalcImpute) for Trainium2, 8 NeuronCores.

Computation (see reference): for each of 8192 receiver rows, find the 16
smallest entries of a 50000-wide distance row (ties -> lowest column index,
matching jax.lax.top_k), gather fit_X_col at those columns, and output the
mean of the valid (mask==0) donor values (0 if none valid).

Sharding: pure data parallel over rows; each of the 8 cores gets 1024 rows.

Device algorithm per 128-row tile (rows live in partitions):
  P1  stream the 50000 columns in 5 panels of 10000, cast f32->bf16 during
      the DMA (SWDGE), then per 80-wide segment compute the min via three
      in-place 2x-mode tensor_tensor min folds (80->40->20->10) plus one
      tensor_reduce (negated) -> nsm = -segmin, 625 f32 per row.
  P2  two rounds of max8/max_index/match_replace on nsm give the 16
      segments with the smallest bf16 minima per row; one more max8 gives
      the 17th seg-min for the coverage flag.
  P3  16 single-offset indirect DMA gathers fetch those segments' original
      f32 values (16x80 per row) from DRAM.
  P4  negate candidates; 2x(max8/find_index8/match_replace) yields the
      indices of the 16 smallest candidates (ties by scan position); a 3rd
      max8 gives the 17th value for the tie-ambiguity flag.

The device returns candidate indices + gathered segment ids + flag; the
host maps them to donor columns and does the (tiny) weighted mean, exactly
reproducing the reference arithmetic in f32.

Exactness: all top-16 values provably live in the 16 segments with the
smallest seg-mins when seg-mins are exact; with bf16(RNE) seg-mins a row
is flagged whenever (17th seg-min scaled down by one bf16 ulp) could reach
the 16th selected value, or the 16th/17th candidates tie. Flagged rows
(plus any with duplicate index reports) are recomputed exactly on host.
"""

import os
import sys

for _p in ("/opt/trn_rl_repo", "/root/.axon_site/_ro/trn_rl_repo"):
    if os.path.isdir(_p) and _p not in sys.path:
        sys.path.insert(0, _p)

import numpy as np

import concourse.bass as bass
import concourse.bacc as bacc_mod
import concourse.mybir as mybir
import concourse.tile as tile
from concourse.bass_utils import run_bass_kernel_spmd

N_CORES = 8
R_TOTAL = 8192
N = 50000
P = 128              # SBUF partitions
S = 80               # segment size for the min prefilter
NSEG = N // S        # 625 segments per row
PC = 10000           # panel columns streamed per DMA
NPAN = N // PC       # 5 panels
SEGP = PC // S       # 125 segments per panel
KSEG = 16            # candidate segments gathered per row
CAND = KSEG * S      # 1280 candidate values per row
NEG_BIG = -3.0e38    # replacement sentinel on the negated scale
BF16_DOWN = 1.0 - 2.0 ** -8   # conservative one-ulp down-scale (values > 0)
F32 = mybir.dt.float32
BF16 = mybir.dt.bfloat16
U32 = mybir.dt.uint32


def build_bass(rows: int, repeat: int = 1):
    """Bass program for one core processing `rows` rows (multiple of 128).

    repeat>1 re-runs the whole pipeline (for slope-based benchmarking).
    """
    assert rows % P == 0
    nt = rows // P

    nc = bacc_mod.Bacc()
    dist = nc.dram_tensor("dist", [rows, N], F32, kind="ExternalInput")
    out_idx = nc.dram_tensor("idx", [P, nt * KSEG], U32, kind="ExternalOutput")
    out_seg = nc.dram_tensor("seg", [P, nt * KSEG], U32, kind="ExternalOutput")
    out_flag = nc.dram_tensor("flag", [P, nt], F32, kind="ExternalOutput")

    # flat view for indirect gathers (offset must be 0)
    dist_flat = dist[:, :].rearrange("r (s e) -> (r s) e", e=S)

    with tile.TileContext(nc) as tc:
        with (
            tc.tile_pool(name="panels", bufs=4) as pan_pool,
            tc.tile_pool(name="segs", bufs=2) as seg_pool,
            tc.tile_pool(name="small", bufs=2) as small_pool,
            tc.tile_pool(name="cands", bufs=2) as cand_pool,
            tc.tile_pool(name="persist", bufs=1) as persist_pool,
        ):
            idx_sb = persist_pool.tile([P, nt, KSEG], U32)
            seg_sb = persist_pool.tile([P, nt, KSEG], U32)
            flag_sb = persist_pool.tile([P, nt], F32)
            rowbase = persist_pool.tile([P, 1], U32)
            nc.gpsimd.iota(rowbase, pattern=[[0, 1]], base=0,
                           channel_multiplier=NSEG)

            def emit_p2(rt, nsm):
                """Top-16 segments of tile rt from negated seg-mins."""
                segidx = small_pool.tile([P, KSEG], U32, tag="segidx")
                v_seg = small_pool.tile([P, 8], F32, tag="v_seg")
                for rnd in range(2):
                    v8 = v_seg[:, :]
                    nc.vector.max(out=v8, in_=nsm)
                    nc.vector.max_index(
                        out=segidx[:, rnd * 8:(rnd + 1) * 8],
                        in_max=v8, in_values=nsm)
                    nc.vector.match_replace(
                        out=nsm, in_to_replace=v8, in_values=nsm,
                        imm_value=NEG_BIG)
                # 17th smallest seg-min (negated), scaled conservatively
                # one bf16 ulp toward zero (nsm is negative).
                m17 = small_pool.tile([P, 8], F32, tag="m17")
                nc.vector.max(out=m17, in_=nsm)
                nc.scalar.mul(m17[:, 0:1], m17[:, 0:1], BF16_DOWN)
                # record gathered segment ids for the host
                nc.vector.tensor_copy(seg_sb[:, rt, :], segidx)
                # offsets into dist_flat: row * NSEG + segidx
                off = small_pool.tile([P, KSEG], U32, tag="off")
                nc.vector.scalar_tensor_tensor(
                    out=off, in0=segidx, scalar=float(rt * P * NSEG),
                    in1=rowbase.to_broadcast([P, KSEG]),
                    op0=mybir.AluOpType.add, op1=mybir.AluOpType.add)
                return dict(rt=rt, off=off, m17=m17)

            def emit_gathers(st):
                cand = cand_pool.tile([P, KSEG, S], F32, tag="cand")
                st["cand"] = cand
                off = st["off"]
                for t in range(KSEG):
                    nc.gpsimd.indirect_dma_start(
                        out=cand[:, t, :], out_offset=None,
                        in_=dist_flat,
                        in_offset=bass.IndirectOffsetOnAxis(
                            ap=off[:, t:t + 1], axis=0),
                    )

            def make_p4_chunks(st):
                """Exact top-16 among candidates, as schedulable chunks."""
                rt, m17 = st["rt"], st["m17"]
                ncand = cand_pool.tile([P, CAND], F32, tag="ncand")
                ncandb = cand_pool.tile([P, CAND], F32, tag="ncandb")
                v_c = small_pool.tile([P, 3, 8], F32, tag="v_c")

                def c1():
                    nc.scalar.mul(
                        ncand, st["cand"].rearrange("p a b -> p (a b)"), -1.0)
                    nc.vector.max(out=v_c[:, 0, :], in_=ncand)
                    nc.vector.max_index(
                        out=idx_sb[:, rt, 0:8], in_max=v_c[:, 0, :],
                        in_values=ncand)

                def c2():
                    nc.vector.match_replace(
                        out=ncandb, in_to_replace=v_c[:, 0, :],
                        in_values=ncand, imm_value=NEG_BIG)
                    nc.vector.max(out=v_c[:, 1, :], in_=ncandb)

                def c3():
                    nc.vector.max_index(
                        out=idx_sb[:, rt, 8:16], in_max=v_c[:, 1, :],
                        in_values=ncandb)
                    nc.vector.match_replace(
                        out=ncandb, in_to_replace=v_c[:, 1, :],
                        in_values=ncandb, imm_value=NEG_BIG)

                def c4():
                    nc.vector.max(out=v_c[:, 2, :], in_=ncandb)
                    # flag = max(v17_cand, m17_seg_scaled) >= v16 (negated
                    # scale): boundary tie or ambiguous segment coverage
                    nc.vector.scalar_tensor_tensor(
                        out=flag_sb[:, rt:rt + 1], in0=v_c[:, 2, 0:1],
                        scalar=m17[:, 0:1], in1=v_c[:, 1, 7:8],
                        op0=mybir.AluOpType.max, op1=mybir.AluOpType.is_ge)

                return [c1, c2, c3, c4]

            def emit_tile(rt, prev):
                """Stream tile rt; interleave prev tile's gathers + P4."""
                chunks = make_p4_chunks(prev) if prev else []
                for pan in range(NPAN):
                    xb = pan_pool.tile([P, SEGP, S], BF16, tag="panel")
                    nc.gpsimd.dma_start(
                        out=xb.rearrange("p s e -> p (s e)"),
                        in_=dist[rt * P:(rt + 1) * P,
                                 pan * PC:(pan + 1) * PC],
                    )
                    if pan == 1 and prev:
                        emit_gathers(prev)
                    if pan == 3 and chunks:
                        chunks.pop(0)()   # c1
                    if pan == 4 and chunks:
                        chunks.pop(0)()   # c2
                    nc.vector.tensor_tensor(
                        out=xb[:, :, 0:40], in0=xb[:, :, 0:40],
                        in1=xb[:, :, 40:80], op=mybir.AluOpType.min)
                    nc.vector.tensor_tensor(
                        out=xb[:, :, 0:20], in0=xb[:, :, 0:20],
                        in1=xb[:, :, 20:40], op=mybir.AluOpType.min)
                    nc.vector.tensor_tensor(
                        out=xb[:, :, 0:10], in0=xb[:, :, 0:10],
                        in1=xb[:, :, 10:20], op=mybir.AluOpType.min)
                    if pan == 0:
                        nsm_t = seg_pool.tile([P, NSEG], F32, tag="nsm")
                        nsm_cur[0] = nsm_t
                    nsm = nsm_cur[0]
                    nc.vector.tensor_reduce(
                        out=nsm[:, pan * SEGP:(pan + 1) * SEGP],
                        in_=xb[:, :, 0:10], axis=mybir.AxisListType.X,
                        op=mybir.AluOpType.min, negate=True)
                while chunks:
                    chunks.pop(0)()       # c3, c4
                return emit_p2(rt, nsm_cur[0])

            nsm_cur = [None]
            prev = None
            for rt in [t for _ in range(repeat) for t in range(nt)]:
                prev = emit_tile(rt, prev)
            emit_gathers(prev)
            for c in make_p4_chunks(prev):
                c()

            nc.sync.dma_start(out=out_idx[:, :],
                              in_=idx_sb.rearrange("p a b -> p (a b)"))
            nc.sync.dma_start(out=out_seg[:, :],
                              in_=seg_sb.rearrange("p a b -> p (a b)"))
            nc.sync.dma_start(out=out_flag[:, :], in_=flag_sb)

    nc.compile()
    return nc


def _host_reference_rows(dist_rows: np.ndarray, fit: np.ndarray,
                         mask: np.ndarray, k: int) -> np.ndarray:
    """Exact recompute (jax.lax.top_k tie semantics) for flagged rows."""
    out = np.empty(dist_rows.shape[0], dtype=np.float32)
    valid = (1 - mask).astype(np.float32)
    for i, row in enumerate(dist_rows):
        r = np.nan_to_num(row, nan=1e10)
        idx = np.argsort(r, kind="stable")[:k]
        w = valid[idx]
        ws = np.float32(w.sum(dtype=np.float32))
        div = ws if ws != 0 else np.float32(1.0)
        num = np.float32((fit[idx].astype(np.float32) * w).sum(dtype=np.float32))
        out[i] = num / div
    return out


def kernel(dist_pot_donors, n_neighbors, fit_X_col, mask_fit_X_col,
           _trace=False, _tmpdir=None):
    dist = np.ascontiguousarray(np.asarray(dist_pot_donors, dtype=np.float32))
    fit = np.asarray(fit_X_col, dtype=np.float32)
    mask = np.asarray(mask_fit_X_col)
    k = int(np.asarray(n_neighbors))
    assert dist.shape == (R_TOTAL, N) and k == 16, (dist.shape, k)

    rows = R_TOTAL // N_CORES
    nt = rows // P

    nc = build_bass(rows)
    in_maps = [{"dist": dist[c * rows:(c + 1) * rows]}
               for c in range(N_CORES)]
    kw = {}
    if _trace:
        kw.update(trace=True, tmpdir=_tmpdir)
    br = run_bass_kernel_spmd(nc, in_maps, core_ids=list(range(N_CORES)), **kw)

    # assemble per-row candidate indices / segment ids / flags
    idx_all = np.empty((R_TOTAL, KSEG), dtype=np.int64)
    seg_all = np.empty((R_TOTAL, KSEG), dtype=np.int64)
    flags = np.empty(R_TOTAL, dtype=bool)
    for c, r in enumerate(br.results):
        # arr[p, t*KSEG + j] holds row c*rows + t*128 + p
        idx = r["idx"].reshape(P, nt, KSEG).transpose(1, 0, 2)
        seg = r["seg"].reshape(P, nt, KSEG).transpose(1, 0, 2)
        fl = r["flag"].T
        idx_all[c * rows:(c + 1) * rows] = idx.reshape(rows, KSEG)
        seg_all[c * rows:(c + 1) * rows] = seg.reshape(rows, KSEG)
        flags[c * rows:(c + 1) * rows] = fl.reshape(rows) != 0

    # duplicate index reports (exact value ties inside the top 16) are
    # ambiguous -> recompute those rows too
    srt = np.sort(idx_all, axis=1)
    flags |= (srt[:, 1:] == srt[:, :-1]).any(axis=1)

    # host finalize: candidate index -> donor column -> weighted mean
    cols = seg_all[np.arange(R_TOTAL)[:, None], idx_all // S] * S + idx_all % S
    valid = (1 - mask).astype(np.float32)
    g = fit * valid
    w = valid[cols]                      # [R, 16]
    ws = w.sum(axis=1, dtype=np.float32)
    num = g[cols].sum(axis=1, dtype=np.float32)
    out = (num / np.where(ws == 0, np.float32(1.0), ws)).astype(np.float32)

    n_flagged = int(flags.sum())
    if n_flagged:
        out[flags] = _host_reference_rows(dist[flags], fit, mask, k)
    kernel._last = {"exec_time_ns": br.exec_time_ns,
                    "mean_exec_time_ns": br.mean_exec_time_ns,
                    "n_flagged": n_flagged,
                    "trace": br.instructions_and_trace}
    return out
